# revision 1
# baseline (speedup 1.0000x reference)
"""Trainium2 Bass kernel for nn_Attention_59949153518227.

Dense transformer block: adaLN-style modulation -> per-stream QKV -> RoPE ->
shared MHA over concat(state, action) -> out_proj -> per-stream MLP with
residual scaling.  B=8 batch elements data-parallel across 8 NeuronCores.

Per-core dataflow (feature-on-partition layout [128p, tiles, tokens]):
  host supplies xT = ((1+scale)*z + shift)^T in bf16 (modulation + transpose
  done host-side); kernel starts dense QKV matmuls immediately.
  xT --matmul wqkvT--> q,k,v  (q,k rows pre-permuted even/odd for RoPE)
  rope(q), rope(k) in-place (elementwise, cos/sin tables from host)
  v' = v.T@wvT + bv  [t, e'] natural, packed per-head with a ones column
  fused attention pipeline per head-pair fo (in_proj | scores | exp | PV):
    q' = wq.T@q (1/8 folded), k' = wk.T@k   [e', t]   (inside the loop)
    scores sT[k,q] = k'_h.T @ q'_h ; p = exp(sT) ; o_h = [v_h|1].T @ p
    row 64 of o = softmax denominator; 1/d via DVE reciprocal_approx_fast
    (no ACT Ln/Exp -> no act-table thrash), rank-1 PE broadcast, normalize.
  y = wo.T@o + bo ; h = gelu(w1.T@y + b1) ; down = h.T@w2T' + b2' (rank-1)
  out = z + down  (residual scale folded into w2/b2 host-side; z fp32)

Matmul dtype is DTM (bfloat16 by default; float32r fallback), fp32 PSUM.
"""
import math
import sys

import numpy as np

try:
    import concourse.bass as bass  # noqa: F401
except ImportError:  # pragma: no cover
    sys.path.insert(0, "/opt/trn_rl_repo")

import ml_dtypes
import concourse.bass as bass
import concourse.tile as tile
from concourse import bacc, mybir
from concourse.bass_utils import run_bass_kernel_spmd

F32 = mybir.dt.float32
F8 = mybir.dt.float8e4
F32R = mybir.dt.float32r
BF16 = mybir.dt.bfloat16
AF = mybir.ActivationFunctionType
OP = mybir.AluOpType

DTM = BF16                      # matmul-side dtype knob: BF16 or F32R
NPM = ml_dtypes.bfloat16 if DTM == BF16 else np.float32

B, S, D, H, HD = 8, 512, 1024, 16, 64
T = 2 * S
FF = 4 * D
P = 128
MAX_LEN = 512.0
N_CORES = 8

_BUILD_CACHE = {}


def _build_nc():
    nc = bacc.Bacc()

    # ---- per-core data inputs ----
    xt_d = [nc.dram_tensor(f"xt{s}", [8, P, S], DTM, kind="ExternalInput") for s in range(2)]
    sz_d = nc.dram_tensor("sz", [S, D], F32, kind="ExternalInput")
    az_d = nc.dram_tensor("az", [S, D], F32, kind="ExternalInput")
    w2T_d = [nc.dram_tensor(f"w2T{s}", [16, P, 2, D], F8, kind="ExternalInput") for s in range(2)]
    b2row_d = [nc.dram_tensor(f"b2row{s}", [1, D], DTM, kind="ExternalInput") for s in range(2)]

    # ---- shared weights/constants (replicated to all cores) ----
    wqkv_d = [nc.dram_tensor(f"wqkv{s}", [24, P, 8, P], DTM, kind="ExternalInput") for s in range(2)]
    bqkv_d = [nc.dram_tensor(f"bqkv{s}", [P, 24], F32, kind="ExternalInput") for s in range(2)]
    wq_d = nc.dram_tensor("wq", [8, P, 8, P], DTM, kind="ExternalInput")
    wk_d = nc.dram_tensor("wk", [8, P, 8, P], DTM, kind="ExternalInput")
    bq_d = nc.dram_tensor("bq", [P, 8], F32, kind="ExternalInput")
    bk_d = nc.dram_tensor("bk", [P, 8], F32, kind="ExternalInput")
    wvT_d = nc.dram_tensor("wvT", [D, D], DTM, kind="ExternalInput")
    bvrow_d = nc.dram_tensor("bvrow", [1, D], DTM, kind="ExternalInput")
    wo_d = nc.dram_tensor("wo", [8, P, 8, P], F8, kind="ExternalInput")
    bo_d = nc.dram_tensor("bo", [P, 8], F32, kind="ExternalInput")
    w1_d = [nc.dram_tensor(f"w1{s}", [32, P, 8, P], F8, kind="ExternalInput") for s in range(2)]
    b1_d = [nc.dram_tensor(f"b1{s}", [P, 32], F32, kind="ExternalInput") for s in range(2)]
    cos_d = nc.dram_tensor("cosT", [P, 2048], DTM, kind="ExternalInput")
    sin_d = nc.dram_tensor("sinT", [P, 2048], DTM, kind="ExternalInput")
    onesr_d = nc.dram_tensor("onesr", [1, P], DTM, kind="ExternalInput")
    onesb_d = nc.dram_tensor("onesb", [P, 512], DTM, kind="ExternalInput")

    out_d = [nc.dram_tensor(n, [S, D], F32, kind="ExternalOutput") for n in ("state_out", "action_out")]

    z_src = [sz_d, az_d]

    with tile.TileContext(nc) as tc:
        with (
            tc.tile_pool(name="big", bufs=8) as big,
            tc.tile_pool(name="pts", bufs=6) as ptsp,
            tc.tile_pool(name="vsbp", bufs=2) as vsbp,
            tc.tile_pool(name="qkp", bufs=6) as qkp,
            tc.tile_pool(name="w1p", bufs=6) as w1p,
            tc.tile_pool(name="w2p", bufs=6) as w2p,
            tc.tile_pool(name="small", bufs=1) as small,
            tc.tile_pool(name="rs", bufs=6) as rs,
            tc.tile_pool(name="rbp", bufs=3) as rbp,
            tc.tile_pool(name="znp", bufs=2) as znp,
            tc.tile_pool(name="psum", bufs=1, space="PSUM") as psum,
        ):
            _ctr = [0]

            def _nm(pfx):
                _ctr[0] += 1
                return f"{pfx}{_ctr[0]}"

            def big_tile(shape):
                return big.tile(shape, DTM, tag="big", name=_nm("bigt"))

            def ps_mm():
                return psum.tile([P, 512], F32, tag="mm", bufs=2, name=_nm("psmm"))

            _rr = [0]

            def dma_w(out, in_):
                # all weight tiles on the SP HWDGE ring: the ACT ring
                # shares the ACT sequencer FIFO, so a slot-waiting DMA
                # there stalls the exp/gelu stream (measured +30us)
                nc.sync.dma_start(out=out, in_=in_)

            def copy_bias(dst, ps, bias_ap):
                # psum -> sbuf with per-partition bias add, on DVE
                nc.vector.tensor_scalar(out=dst, in0=ps, scalar1=bias_ap, scalar2=None, op0=OP.add)

            def copy_bias_act(dst, ps, bias_ap):
                # same, on ACT (used where ACT is otherwise idle and DVE busy)
                nc.scalar.activation(dst, ps, AF.Identity, bias=bias_ap)

            # ---- xT tiles (host pre-modulated/transposed), one DMA each.
            # xt0 leads the SP ring; xt1 is emitted inside the QKV loop so
            # stream-0's weight tiles aren't queued behind it. ----
            xTt = [None, None]

            def load_xt(s):
                t_ = znp.tile([P, 8, S], DTM, tag="zn", name=_nm("xt"))
                nc.sync.dma_start(out=t_[:], in_=xt_d[s].rearrange("o p t -> p o t"))
                xTt[s] = t_

            load_xt(0)

            # ---- constants (ACT HWDGE ring; keeps the SP ring clear for
            # xt chunks + weight tiles so QKV matmuls start ASAP) ----
            onesr = small.tile([1, P], DTM)
            nc.scalar.dma_start(out=onesr[:], in_=onesr_d[:])
            bqkv = []
            for s in range(2):
                t_ = small.tile([P, 24], F32, tag=f"bqkv{s}")
                nc.scalar.dma_start(out=t_[:], in_=bqkv_d[s][:])
                bqkv.append(t_)
            bq = small.tile([P, 8], F32, tag="bq")
            nc.scalar.dma_start(out=bq[:], in_=bq_d[:])
            bk = small.tile([P, 8], F32, tag="bk")
            nc.scalar.dma_start(out=bk[:], in_=bk_d[:])
            bo = small.tile([P, 8], F32, tag="bo")
            nc.scalar.dma_start(out=bo[:], in_=bo_d[:])
            onesb = small.tile([P, 512], DTM, tag="onesb")
            nc.scalar.dma_start(out=onesb[:], in_=onesb_d[:])
            bvrow = small.tile([1, D], DTM, tag="bvrow")
            nc.scalar.dma_start(out=bvrow[:], in_=bvrow_d[:])
            b1 = []
            for s in range(2):
                t_ = small.tile([P, 32], F32, tag=f"b1{s}")
                nc.scalar.dma_start(out=t_[:], in_=b1_d[s][:])
                b1.append(t_)
            b2row = []
            for s in range(2):
                t_ = small.tile([1, D], DTM, tag=f"b2row{s}")
                nc.scalar.dma_start(out=t_[:], in_=b2row_d[s][:])
                b2row.append(t_)
            tbl = big_tile([P, 4096])
            nc.scalar.dma_start(out=tbl[:, 0:2048], in_=cos_d[:])
            nc.scalar.dma_start(out=tbl[:, 2048:4096], in_=sin_d[:])

            # v_sb ones columns via DVE memset (a strided DMA here costs
            # ~2-5us of descriptor time each and stalls the queue)
            vsb = [vsbp.tile([P, 8, 8, 65], DTM, tag="vsb", name=_nm("vsb")) for _ in range(2)]
            for ec in range(2):
                nc.vector.memset(vsb[ec][:, :, :, 64:65], 1.0)

            # ---- stage C: rope on q and k blocks, in place (per stream;
            # emitted right after that stream's QKV so DVE overlaps the
            # other stream's matmuls) ----
            def rope_stream(s):
                for j in range(2):
                    tgt = qkv[s][j]
                    for i in range(4):
                        qe = tgt[:, i, :]
                        qo = tgt[:, 4 + i, :]
                        cos_i = tbl[:, i * 512:(i + 1) * 512]
                        sin_i = tbl[:, 2048 + i * 512:2048 + (i + 1) * 512]
                        m1 = rs.tile([P, 512], DTM, tag="rs", name=_nm("rst"))
                        m2 = rs.tile([P, 512], DTM, tag="rs", name=_nm("rst"))
                        m3 = rs.tile([P, 512], DTM, tag="rs", name=_nm("rst"))
                        m4 = rs.tile([P, 512], DTM, tag="rs", name=_nm("rst"))
                        nc.vector.tensor_tensor(m1[:], qe, cos_i, OP.mult)
                        nc.vector.tensor_tensor(m2[:], qo, sin_i, OP.mult)
                        nc.vector.tensor_tensor(m3[:], qe, sin_i, OP.mult)
                        nc.vector.tensor_tensor(m4[:], qo, cos_i, OP.mult)
                        nc.vector.tensor_tensor(tgt[:, i, :], m1[:], m2[:], OP.subtract)
                        nc.vector.tensor_tensor(tgt[:, 4 + i, :], m3[:], m4[:], OP.add)

            # ---- stage B: per-stream QKV ----
            qkv = []  # [stream][j] j=0 q, 1 k, 2 v ; each [128, 8, 512]
            for s in range(2):
                parts = [big_tile([P, 8, S]) for _ in range(3)]
                for eo in range(24):
                    wt = w1p.tile([P, 8, P], DTM, tag="w1", name=_nm("wt"))
                    dma_w(wt[:], wqkv_d[s][eo])
                    ps = ps_mm()
                    for ko in range(8):
                        nc.tensor.matmul(ps[:], lhsT=wt[:, ko, :], rhs=xTt[s][:, ko, :],
                                         start=(ko == 0), stop=(ko == 7))
                    j, col = divmod(eo, 8)
                    copy_bias_act(parts[j][:, col, :], ps[:], bqkv[s][:, eo:eo + 1])
                qkv.append(parts)
                rope_stream(s)
                if s == 0:
                    load_xt(1)

            # ---- stage D: v' in natural [t, e'] layout, per-head + ones col.
            # 4 token-groups per wt2 pass (psum tags mm,mm,sc,pv like
            # mlp_down) so each wvT tile is DMA'd only twice, not 4x.
            def vprime_chunk(ec, tog):
                pss = [ps_mm(), ps_mm(),
                       psum.tile([P, 512], F32, tag="sc", bufs=2, name=_nm("psv")),
                       psum.tile([P, 512], F32, tag="pv", bufs=2, name=_nm("psv"))]
                for vo in range(8):
                    wt2 = w2p.tile([P, 512], DTM, tag="w2", name=_nm("wt2"))
                    dma_w(wt2[:], wvT_d[vo * P:(vo + 1) * P, ec * 512:(ec + 1) * 512])
                    for tl in range(4):
                        tg = tog * 4 + tl
                        s2, ttt = divmod(tg, 4)
                        nc.tensor.matmul(pss[tl][:], lhsT=qkv[s2][2][:, vo, ttt * P:(ttt + 1) * P],
                                         rhs=wt2[:], start=(vo == 0), stop=False)
                for tl in range(4):
                    nc.tensor.matmul(pss[tl][:], lhsT=onesr[:], rhs=bvrow[:, ec * 512:(ec + 1) * 512],
                                     start=False, stop=True)
                for tl in range(4):
                    kt = tog * 4 + tl
                    nc.vector.tensor_copy(vsb[ec][:, kt, :, 0:64],
                                          pss[tl][:].rearrange("p (h c) -> p h c", h=8))

            for ec in range(2):
                for tog in range(2):
                    vprime_chunk(ec, tog)

            # ---- fused attention: per head-pair fo, both query streams ----
            # in_proj(fo+1) | scores(fo) | exp(fo) | PV(fo) | tails(fo)
            # PE stays dense while ACT streams the exps.
            oT = [big_tile([P, 8, S]) for _ in range(2)]
            oT8 = [big.tile([P, 8, S], F8, tag="big", name=_nm("o8")) for _ in range(2)]
            pending = []

            def in_proj_fo(fo):
                qf = qkp.tile([P, T], DTM, tag="qk", name=_nm("qf"))
                kf = qkp.tile([P, T], DTM, tag="qk", name=_nm("kf"))
                for jj, wd, bb, dst in ((0, wq_d, bq, qf), (1, wk_d, bk, kf)):
                    wt = w1p.tile([P, 8, P], DTM, tag="w1", name=_nm("wt"))
                    dma_w(wt[:], wd[fo])
                    for qc in range(2):
                        ps = ps_mm()
                        for ko in range(8):
                            nc.tensor.matmul(ps[:], lhsT=wt[:, ko, :], rhs=qkv[qc][jj][:, ko, :],
                                             start=(ko == 0), stop=(ko == 7))
                        copy_bias(dst[:, qc * S:(qc + 1) * S], ps[:], bb[:, fo:fo + 1])
                return qf, kf

            def scores_fo(fo, qc, qf, kf):
                pTs2 = [ptsp.tile([P, 8, S], DTM, tag="pts", name=_nm("pts"))
                        for _ in range(2)]
                for kcp in range(4):
                    pp = [psum.tile([P, T], F32, tag="sc", bufs=2, name=_nm("pssc"))
                          for _ in range(2)]
                    for ki in range(2):
                        kc = 2 * kcp + ki
                        for hp in range(2):
                            poff = 64 * hp
                            nc.tensor.matmul(
                                pp[hp][:, ki * S:(ki + 1) * S],
                                lhsT=kf[poff:poff + 64, kc * P:(kc + 1) * P],
                                rhs=qf[poff:poff + 64, qc * S:(qc + 1) * S],
                                start=True, stop=True, tile_position=(poff, 0))
                    for hp in range(2):
                        nc.scalar.activation(pTs2[hp][:, 2 * kcp:2 * kcp + 2, :], pp[hp][:], AF.Exp)
                return pTs2

            def pv_fo(fo, qc, pTs2):
                for hp in range(2):
                    h = 2 * fo + hp
                    vt = vsb[h // 8]
                    hh = h % 8
                    op = psum.tile([P, 512], F32, tag="pv", bufs=2, name=_nm("pspv"))
                    for kc in range(8):
                        nc.tensor.matmul(op[0:65, :], lhsT=vt[:, kc, hh, :],
                                         rhs=pTs2[hp][:, kc, :], start=(kc == 0), stop=(kc == 7))
                    pending.append((qc, fo, 64 * hp, op))

            # Softmax denominators: groups of 3 tails gathered into one
            # [65,512] tile at partitions {64,32,0} (row 64 straight from
            # psum on DVE; rows 32/0 via tiny SWDGE SB2SB partition-move),
            # ONE native DVE reciprocal per group (lane-parallel), then a
            # rank-1 PE broadcast per head.  {0,32,64} are the only legal
            # matmul base partitions, which sets the group size.
            gbuf = {"dall": None, "list": []}
            _ROWS = (64, 32, 0)

            def flush_group():
                lst = gbuf["list"]
                if not lst:
                    return
                dall = gbuf["dall"]
                rc = rbp.tile([65, 512], F32, tag="rc", name=_nm("rc"), bufs=2)
                nc.vector.reciprocal(rc[:], dall[:])
                rcb = rbp.tile([65, 512], DTM, tag="rcb", name=_nm("rcb"), bufs=1)
                with nc.allow_low_precision(reason="softmax 1/denom to bf16"):
                    nc.vector.tensor_copy(rcb[:], rc[:])
                for (qc_, fo_, poff_, op_), r in lst:
                    bp = psum.tile([P, 512], F32, tag="mm", bufs=2, name=_nm("psbc"))
                    nc.tensor.matmul(bp[0:64, :], lhsT=onesb[r:r + 1, 0:64], rhs=rcb[r:r + 1, :],
                                     start=True, stop=True)
                    with nc.allow_low_precision(reason="normalized o to fp8 for DoubleRow out_proj"):
                        nc.vector.tensor_tensor(oT8[qc_][poff_:poff_ + 64, fo_, :],
                                                oT[qc_][poff_:poff_ + 64, fo_, :], bp[0:64, :], OP.mult)
                gbuf["dall"] = None
                gbuf["list"] = []

            def tail_push(ent):
                qc_, fo_, poff_, op_ = ent
                nc.vector.tensor_copy(oT[qc_][poff_:poff_ + 64, fo_, :], op_[0:64, :])
                if gbuf["dall"] is None:
                    gbuf["dall"] = rbp.tile([65, 512], F32, tag="dall", name=_nm("dall"), bufs=2)
                r = _ROWS[len(gbuf["list"])]
                if r == 64:
                    nc.vector.tensor_copy(gbuf["dall"][64:65, :], op_[64:65, :])
                else:
                    dsb = rbp.tile([65, 512], F32, tag="dsb", name=_nm("dsb"), bufs=2)
                    nc.vector.tensor_copy(dsb[64:65, :], op_[64:65, :])
                    nc.gpsimd.dma_start(out=gbuf["dall"][r:r + 1, :], in_=dsb[64:65, :])
                gbuf["list"].append((ent, r))
                if len(gbuf["list"]) == 3:
                    flush_group()

            def tail_flush():
                flush_group()

            # two-iteration in_proj lookahead: keeps ready PE work queued
            # while the exp chain gates scores near fo boundaries
            ipj = in_proj_fo(0)
            ipj2 = in_proj_fo(1)
            for fo in range(8):
                qf, kf = ipj
                cur = [(qc, scores_fo(fo, qc, qf, kf)) for qc in range(2)]
                ipj = ipj2
                if fo < 6:
                    ipj2 = in_proj_fo(fo + 2)
                for qc, pTs2 in cur:
                    pv_fo(fo, qc, pTs2)
                    for ent in pending:
                        tail_push(ent)
                    pending = []
            tail_flush()

            # ---- out_proj + per-stream MLP, dense PE ----
            def out_proj(qc, yq):
                for eo in range(8):
                    wt = w1p.tile([P, 8, P], F8, tag="w1", name=_nm("wt"))
                    dma_w(wt[:], wo_d[eo])
                    ps = ps_mm()
                    for fp in range(4):
                        nc.tensor.matmul(ps[:], lhsT=wt[:, 2 * fp:2 * fp + 2, :],
                                         rhs=oT8[qc][:, 2 * fp:2 * fp + 2, :],
                                         start=(fp == 0), stop=(fp == 3),
                                         perf_mode=mybir.MatmulPerfMode.DoubleRow)
                    with nc.allow_low_precision(reason="y to fp8 for DoubleRow up"):
                        nc.vector.tensor_scalar(out=yq[:, eo, :], in0=ps[:], scalar1=0.125,
                                                scalar2=bo[:, eo:eo + 1], op0=OP.mult, op1=OP.add)

            def mlp_up_group(s, fo, yq, hts):
                wt = w1p.tile([P, 8, P], F8, tag="w1", name=_nm("wt"))
                dma_w(wt[:], w1_d[s][fo])
                ps = ps_mm()
                for kp in range(4):
                    nc.tensor.matmul(ps[:], lhsT=wt[:, 2 * kp:2 * kp + 2, :],
                                     rhs=yq[:, 2 * kp:2 * kp + 2, :],
                                     start=(kp == 0), stop=(kp == 3),
                                     perf_mode=mybir.MatmulPerfMode.DoubleRow)
                with nc.allow_low_precision(reason="mlp hidden to fp8 for DoubleRow down"):
                    nc.scalar.activation(hts[fo // 8][:, fo % 8, :], ps[:], AF.Gelu_apprx_tanh,
                                         bias=b1[s][:, fo:fo + 1], scale=0.125)

            def mlp_down_ec(s, hts, ec):
                zn2 = znp.tile([P, 4, 512], F32, tag="zn", name=_nm("zn2"))
                nc.sync.dma_start(
                    out=zn2[:],
                    in_=z_src[s][:, ec * 512:(ec + 1) * 512].rearrange("(to p) d -> p to d", p=P))
                pss = [ps_mm(), ps_mm(),
                       psum.tile([P, 512], F32, tag="sc", bufs=2, name=_nm("psg")),
                       psum.tile([P, 512], F32, tag="pv", bufs=2, name=_nm("psg"))]
                # fp8 DoubleRow: both operands 3D [128, 2, X] = two ff-chunk
                # k-tiles per pass; w2 host-scaled by 8 (e4m3 subnormal
                # floor), un-done in the fused epilogue below
                for cp in range(16):
                    wt2 = w2p.tile([P, 2, 512], F8, tag="w2", name=_nm("wt2"))
                    dma_w(wt2[:], w2T_d[s][cp][:, :, ec * 512:(ec + 1) * 512])
                    fo = 2 * cp
                    for tl in range(4):
                        nc.tensor.matmul(pss[tl][:],
                                         lhsT=hts[fo // 8][:, fo % 8:fo % 8 + 2, tl * P:(tl + 1) * P],
                                         rhs=wt2[:], start=(cp == 0), stop=False,
                                         perf_mode=mybir.MatmulPerfMode.DoubleRow)
                for tl in range(4):
                    nc.tensor.matmul(pss[tl][:], lhsT=onesr[:], rhs=b2row[s][:, ec * 512:(ec + 1) * 512],
                                     start=False, stop=True)
                for tl in range(4):
                    t2 = rs.tile([P, 512], F32, tag="rs", name=_nm("ost"))
                    nc.vector.scalar_tensor_tensor(t2[:], pss[tl][:], 0.125, zn2[:, tl, :],
                                                   OP.mult, OP.add)
                    nc.sync.dma_start(out=out_d[s][tl * P:(tl + 1) * P, ec * 512:(ec + 1) * 512], in_=t2[:])

            yT = {}
            for qc in (0, 1):
                y = big.tile([P, 8, S], F8, tag="big", name=_nm("yf"))
                out_proj(qc, y)
                yT[qc] = y
            for s in (0, 1):
                hts = [big.tile([P, 8, S], F8, tag="big", name=_nm("htf")) for _ in range(4)]
                for fo in range(32):
                    mlp_up_group(s, fo, yT[s], hts)
                mlp_down_ec(s, hts, 0)
                mlp_down_ec(s, hts, 1)

    nc.finalize()
    return nc


def _to4(WT):
    """WT [Din, Eout] -> [Eout/128, 128p, Din/128, 128e] tiles for lhsT DMA."""
    din, eout = WT.shape
    a = WT.reshape(din // P, P, eout // P, P)       # [ko, p, eo, e]
    return np.ascontiguousarray(a.transpose(2, 1, 0, 3).astype(NPM))


def _bias_part(b, n_tiles):
    return np.ascontiguousarray(b.reshape(n_tiles, P).T)


def _prep_shared(inputs):
    f32 = lambda x: np.ascontiguousarray(np.asarray(x, dtype=np.float32))
    perm = np.concatenate([np.arange(0, D, 2), np.arange(1, D, 2)])

    shared = {}
    for s, (wn, bn) in enumerate((("qkv_state_w", "qkv_state_b"), ("qkv_action_w", "qkv_action_b"))):
        w = f32(inputs[wn])
        b = f32(inputs[bn])
        wp = np.concatenate([w[0:D][perm], w[D:2 * D][perm], w[2 * D:3 * D]], axis=0)
        bp = np.concatenate([b[0:D][perm], b[D:2 * D][perm], b[2 * D:3 * D]])
        shared[f"wqkv{s}"] = _to4(wp.T)
        shared[f"bqkv{s}"] = _bias_part(bp, 24)

    in_w = f32(inputs["attn_in_w"])
    in_b = f32(inputs["attn_in_b"])
    wq, wk, wv = in_w[0:D], in_w[D:2 * D], in_w[2 * D:3 * D]
    bq_, bk_, bv_ = in_b[0:D], in_b[D:2 * D], in_b[2 * D:3 * D]
    scale = 1.0 / math.sqrt(HD)
    shared["wq"] = _to4((wq[:, perm] * scale).T)
    shared["bq"] = _bias_part(bq_ * scale, 8)
    shared["wk"] = _to4(wk[:, perm].T)
    shared["bk"] = _bias_part(bk_, 8)
    shared["wvT"] = np.ascontiguousarray(wv.T.astype(NPM))
    shared["bvrow"] = np.ascontiguousarray(bv_[None, :].astype(NPM))
    shared["wo"] = np.ascontiguousarray(
        (_to4(f32(inputs["attn_out_w"]).T).astype(np.float32) * 8.0).astype(mybir.dt.np(F8)))
    shared["bo"] = _bias_part(f32(inputs["attn_out_b"]), 8)
    np8 = mybir.dt.np(F8)
    for s, pre in enumerate(("mlp_state", "mlp_action")):
        w1a = _to4(f32(inputs[f"{pre}_w1"]).T).astype(np.float32) * 8.0
        shared[f"w1{s}"] = np.ascontiguousarray(w1a.astype(np8))
        shared[f"b1{s}"] = _bias_part(f32(inputs[f"{pre}_b1"]), 32)

    inv = np.exp(-math.log(MAX_LEN) * np.arange(0, D, 2, dtype=np.float64) / D)
    theta = inv[:, None] * np.arange(S, dtype=np.float64)[None, :]   # [i, t]
    cosT = np.cos(theta).astype(np.float32)
    sinT = np.sin(theta).astype(np.float32)
    shared["cosT"] = np.ascontiguousarray(cosT.reshape(4, P, S).transpose(1, 0, 2).reshape(P, 2048).astype(NPM))
    shared["sinT"] = np.ascontiguousarray(sinT.reshape(4, P, S).transpose(1, 0, 2).reshape(P, 2048).astype(NPM))
    shared["onesr"] = np.ones((1, P), NPM)
    shared["onesb"] = np.ones((P, 512), NPM)
    return shared


def _prep_in_maps(inputs):
    f32 = lambda x: np.ascontiguousarray(np.asarray(x, dtype=np.float32))
    shared = _prep_shared(inputs)
    state_z = f32(inputs["state_z"])
    action_z = f32(inputs["action_z"])
    e = f32(inputs["e"])
    w2T = [f32(inputs["mlp_state_w2"]).T, f32(inputs["mlp_action_w2"]).T]   # [FF, D]
    b2 = [f32(inputs["mlp_state_b2"]), f32(inputs["mlp_action_b2"])]
    in_maps = []
    for b in range(B):
        shift = e[b, 0, 0:D]
        scl1p = 1.0 + e[b, 0, D:2 * D]
        res = e[b, 0, 2 * D:3 * D]
        m = dict(shared)
        m["sz"] = state_z[b]
        m["az"] = action_z[b]
        np8 = mybir.dt.np(F8)
        for s, z in ((0, state_z[b]), (1, action_z[b])):
            xt = (scl1p[None, :] * z + shift[None, :]).T      # [D, S]
            m[f"xt{s}"] = np.ascontiguousarray(xt.reshape(8, P, S).astype(NPM))
            w2r = (w2T[s] * res[None, :] * 8.0).reshape(16, 2, P, D).transpose(0, 2, 1, 3)
            m[f"w2T{s}"] = np.ascontiguousarray(w2r.astype(np8))
            m[f"b2row{s}"] = np.ascontiguousarray((b2[s] * res * 8.0)[None, :].astype(NPM))
        in_maps.append(m)
    return in_maps


def _run(inputs, trace=False, trace_kwargs=None, tmpdir=None):
    key = "nc"
    if key not in _BUILD_CACHE:
        _BUILD_CACHE[key] = _build_nc()
    nc = _BUILD_CACHE[key]
    in_maps = _prep_in_maps(inputs)
    kw = {}
    if trace:
        kw = dict(trace=True, trace_kwargs=trace_kwargs or {})
    if tmpdir is not None:
        kw["tmpdir"] = tmpdir
    return run_bass_kernel_spmd(nc, in_maps, list(range(N_CORES)), **kw)


def kernel(**inputs):
    res = _run(inputs)
    state = np.stack([res.results[b]["state_out"] for b in range(B)])
    action = np.stack([res.results[b]["action_out"] for b in range(B)])
    return (state, action)


def kernel_timed(tmpdir=None, **inputs):
    """Returns ((state, action), exec_time_ns) using the NTFF profile path."""
    res = _run(inputs, trace=True, tmpdir=tmpdir)
    state = np.stack([res.results[b]["state_out"] for b in range(B)])
    action = np.stack([res.results[b]["action_out"] for b in range(B)])
    return (state, action), res.exec_time_ns



# revision 19
# speedup vs baseline: 1.2811x; 1.2811x over previous
"""Trainium2 Bass kernel for nn_Attention_59949153518227.

Dense transformer block: adaLN-style modulation -> per-stream QKV -> RoPE ->
shared MHA over concat(state, action) -> out_proj -> per-stream MLP with
residual scaling.  B=8 batch elements data-parallel across 8 NeuronCores.

v2 layout (vs v1 baseline at ~568us):
  - QKV, attention in_proj (wq/wk) and v' (wv) matmuls run fp8 DoubleRow
    (x, roped q/k, and v quantized to e4m3; weights x8-scaled e4m3, descale
    folded into epilogues).  Numpy sim puts the absmax-rel cost of this at
    +0.8e-3 over the v1 quantization (MLP fp8 chain dominates the budget).
  - Phase restructure: QKV computes q,k first, ropes them (rope writes the
    fp8 copies directly), and in_proj/scores/exp start while the v-part
    QKV, v' and remaining weights stream in as low-priority PE filler
    inside the attention loop.  ACT (exp stream, ~147us) and PE are
    co-saturated through the attention phase instead of serializing.
  - Softmax denominator: PV keeps the [v|1] ones-column trick (M=65); the
    per-head denominator row goes through reciprocal_approx_fast (~5x
    faster than reciprocal) straight out of PSUM at partition 64, then a
    rank-1 PE broadcast -- no cross-partition gather, no gpsimd moves.
  - exp stays exclusively on ACT during attention; all v-path epilogues on
    DVE.

Matmul dtype: fp8 DoubleRow for all dense projections; bf16 for scores/PV.
"""
import math
import sys

import numpy as np

try:
    import concourse.bass as bass  # noqa: F401
except ImportError:  # pragma: no cover
    sys.path.insert(0, "/opt/trn_rl_repo")

import ml_dtypes
import concourse.bass as bass
import concourse.tile as tile
from concourse import bacc, mybir
from concourse.bass_utils import run_bass_kernel_spmd

F32 = mybir.dt.float32
F8 = mybir.dt.float8e4
BF16 = mybir.dt.bfloat16
AF = mybir.ActivationFunctionType
OP = mybir.AluOpType
DR = mybir.MatmulPerfMode.DoubleRow

DTM = BF16
NPM = ml_dtypes.bfloat16

B, S, D, H, HD = 8, 512, 1024, 16, 64
T = 2 * S
FF = 4 * D
P = 128
MAX_LEN = 512.0
N_CORES = 8

_BUILD_CACHE = {}


def _build_nc(debug=False):
    nc = bacc.Bacc()
    np8 = mybir.dt.np(F8)
    dbg = {}

    def dbg_dump(name, ap, shape, dtype):
        if not debug:
            return
        t = nc.dram_tensor(name, shape, dtype, kind="ExternalOutput")
        nc.sync.dma_start(out=t[:], in_=ap)
        dbg[name] = t

    # ---- per-core data inputs ----
    xt_d = [nc.dram_tensor(f"xt{s}", [8, P, S], F8, kind="ExternalInput") for s in range(2)]
    sz_d = nc.dram_tensor("sz", [S, D], F32, kind="ExternalInput")
    az_d = nc.dram_tensor("az", [S, D], F32, kind="ExternalInput")
    w2T_d = [nc.dram_tensor(f"w2T{s}", [16, P, 2, D], F8, kind="ExternalInput") for s in range(2)]
    b2row_d = [nc.dram_tensor(f"b2row{s}", [1, D], DTM, kind="ExternalInput") for s in range(2)]

    # ---- shared weights/constants (replicated to all cores) ----
    wqkv_d = [nc.dram_tensor(f"wqkv{s}", [24, P, 8, P], F8, kind="ExternalInput") for s in range(2)]
    bqkv_d = [nc.dram_tensor(f"bqkv{s}", [P, 24], F32, kind="ExternalInput") for s in range(2)]
    wq_d = nc.dram_tensor("wq", [8, P, 8, P], F8, kind="ExternalInput")
    wk_d = nc.dram_tensor("wk", [8, P, 8, P], F8, kind="ExternalInput")
    bq_d = nc.dram_tensor("bq", [P, 8], F32, kind="ExternalInput")   # pre-scaled by 1/8
    bk_d = nc.dram_tensor("bk", [P, 8], F32, kind="ExternalInput")
    wvT_d = nc.dram_tensor("wvT", [4, P, 2, D], F8, kind="ExternalInput")
    bvrow_d = nc.dram_tensor("bvrow", [1, D], DTM, kind="ExternalInput")  # pre-scaled by 8
    wo_d = nc.dram_tensor("wo", [8, P, 8, P], F8, kind="ExternalInput")
    bo_d = nc.dram_tensor("bo", [P, 8], F32, kind="ExternalInput")
    w1_d = [nc.dram_tensor(f"w1{s}", [32, P, 8, P], F8, kind="ExternalInput") for s in range(2)]
    b1_d = [nc.dram_tensor(f"b1{s}", [P, 32], F32, kind="ExternalInput") for s in range(2)]
    cos_d = nc.dram_tensor("cosT", [P, 2048], DTM, kind="ExternalInput")
    sin_d = nc.dram_tensor("sinT", [P, 2048], DTM, kind="ExternalInput")
    onesr_d = nc.dram_tensor("onesr", [1, P], DTM, kind="ExternalInput")
    onesb_d = nc.dram_tensor("onesb", [P, 512], DTM, kind="ExternalInput")

    out_d = [nc.dram_tensor(n, [S, D], F32, kind="ExternalOutput") for n in ("state_out", "action_out")]

    z_src = [sz_d, az_d]

    with tile.TileContext(nc) as tc:
        with (
            tc.tile_pool(name="big", bufs=1) as big,
            tc.tile_pool(name="pts", bufs=6) as ptsp,
            tc.tile_pool(name="vsbp", bufs=2) as vsbp,
            tc.tile_pool(name="qkp", bufs=6) as qkp,
            tc.tile_pool(name="w1p", bufs=6) as w1p,
            tc.tile_pool(name="w2p", bufs=6) as w2p,
            tc.tile_pool(name="small", bufs=1) as small,
            tc.tile_pool(name="rs", bufs=6) as rs,
            tc.tile_pool(name="rbp", bufs=2) as rbp,
            tc.tile_pool(name="znp", bufs=2) as znp,
            tc.tile_pool(name="psum", bufs=1, space="PSUM") as psum,
        ):
            _ctr = [0]

            def _nm(pfx):
                _ctr[0] += 1
                return f"{pfx}{_ctr[0]}"

            def ps_mm():
                return psum.tile([P, 512], F32, tag="mm", bufs=2, name=_nm("psmm"))

            def dma_w(out, in_):
                # weight tiles ride the SP HWDGE ring; ACT ring is kept for
                # constants so the exp stream's sequencer FIFO stays clear
                nc.sync.dma_start(out=out, in_=in_)

            # ---- xT tiles (host pre-modulated/transposed/fp8) ----
            xTt = [None, None]

            def load_xt(s):
                t_ = znp.tile([P, 8, S], F8, tag="xt", bufs=2, name=_nm("xt"))
                nc.sync.dma_start(out=t_[:], in_=xt_d[s].rearrange("o p t -> p o t"))
                xTt[s] = t_

            load_xt(0)

            # ---- constants (ACT HWDGE ring) ----
            onesr = small.tile([1, P], DTM)
            nc.scalar.dma_start(out=onesr[:], in_=onesr_d[:])
            bqkv = []
            for s in range(2):
                t_ = small.tile([P, 24], F32, tag=f"bqkv{s}")
                nc.scalar.dma_start(out=t_[:], in_=bqkv_d[s][:])
                bqkv.append(t_)
            bq = small.tile([P, 8], F32, tag="bq")
            nc.scalar.dma_start(out=bq[:], in_=bq_d[:])
            bk = small.tile([P, 8], F32, tag="bk")
            nc.scalar.dma_start(out=bk[:], in_=bk_d[:])
            bo = small.tile([P, 8], F32, tag="bo")
            nc.scalar.dma_start(out=bo[:], in_=bo_d[:])
            onesb = small.tile([P, 512], DTM, tag="onesb")
            nc.scalar.dma_start(out=onesb[:], in_=onesb_d[:])
            bvrow = small.tile([1, D], DTM, tag="bvrow")
            nc.scalar.dma_start(out=bvrow[:], in_=bvrow_d[:])
            b1 = []
            for s in range(2):
                t_ = small.tile([P, 32], F32, tag=f"b1{s}")
                nc.scalar.dma_start(out=t_[:], in_=b1_d[s][:])
                b1.append(t_)
            b2row = []
            for s in range(2):
                t_ = small.tile([1, D], DTM, tag=f"b2row{s}")
                nc.scalar.dma_start(out=t_[:], in_=b2row_d[s][:])
                b2row.append(t_)
            tbl = big.tile([P, 4096], DTM, tag="tbl", bufs=1, name="tbl")
            nc.scalar.dma_start(out=tbl[:, 0:2048], in_=cos_d[:])
            nc.scalar.dma_start(out=tbl[:, 2048:4096], in_=sin_d[:])

            # long-lived SBUF tensors, each tag sized to its live set
            qkv = [[big.tile([P, 8, S], DTM, tag="qkvqk", bufs=4, name=_nm("qk"))
                    for _ in range(2)] for _ in range(2)]     # [s][j] bf16 q,k
            qk8 = [[big.tile([P, 8, S], F8, tag="qk8", bufs=4, name=_nm("qk8"))
                    for _ in range(2)] for _ in range(2)]     # fp8 roped q,k
            v8 = [big.tile([P, 8, S], F8, tag="v8", bufs=2, name=_nm("v8"))
                  for _ in range(2)]                          # fp8 v per stream
            vsb = [vsbp.tile([P, 8, 8, 65], DTM, tag="vsb", name=_nm("vsb")) for _ in range(2)]
            for ec in range(2):
                nc.vector.memset(vsb[ec][:, :, :, 64:65], 1.0)
            # oT reuses the bf16 q/k ring (dead after rope)
            oT = [big.tile([P, 8, S], DTM, tag="qkvqk", bufs=4, name=_nm("oT")) for _ in range(2)]
            oT8 = [big.tile([P, 8, S], F8, tag="oT8", bufs=2, name=_nm("o8")) for _ in range(2)]

            # ---- QKV (fp8 DoubleRow) ----
            def qkv_block(s, eos):
                for eo in eos:
                    wt = w1p.tile([P, 8, P], F8, tag="w1", name=_nm("wt"))
                    dma_w(wt[:], wqkv_d[s][eo])
                    ps = ps_mm()
                    for kp in range(4):
                        nc.tensor.matmul(ps[:], lhsT=wt[:, 2 * kp:2 * kp + 2, :],
                                         rhs=xTt[s][:, 2 * kp:2 * kp + 2, :],
                                         start=(kp == 0), stop=(kp == 3), perf_mode=DR)
                    j, col = divmod(eo, 8)
                    if j < 2:
                        # q,k -> bf16 (rope input); ACT is idle pre-attention
                        nc.scalar.activation(qkv[s][j][:, col, :], ps[:], AF.Identity,
                                             bias=bqkv[s][:, eo:eo + 1], scale=0.125)
                    else:
                        with nc.allow_low_precision(reason="v to fp8 for DR v-proj"):
                            nc.vector.tensor_scalar(out=v8[s][:, col, :], in0=ps[:],
                                                    scalar1=0.125, scalar2=bqkv[s][:, eo:eo + 1],
                                                    op0=OP.mult, op1=OP.add)

            # ---- rope: bf16 q/k in, fp8 out (for DR in_proj) ----
            def rope_stream(s):
                for j in range(2):
                    src = qkv[s][j]
                    dst = qk8[s][j]
                    for i in range(4):
                        qe = src[:, i, :]
                        qo = src[:, 4 + i, :]
                        cos_i = tbl[:, i * 512:(i + 1) * 512]
                        sin_i = tbl[:, 2048 + i * 512:2048 + (i + 1) * 512]
                        m1 = rs.tile([P, 512], DTM, tag="rs", name=_nm("rst"))
                        m2 = rs.tile([P, 512], DTM, tag="rs", name=_nm("rst"))
                        m3 = rs.tile([P, 512], DTM, tag="rs", name=_nm("rst"))
                        m4 = rs.tile([P, 512], DTM, tag="rs", name=_nm("rst"))
                        nc.vector.tensor_tensor(m1[:], qe, cos_i, OP.mult)
                        nc.vector.tensor_tensor(m2[:], qo, sin_i, OP.mult)
                        nc.vector.tensor_tensor(m3[:], qe, sin_i, OP.mult)
                        nc.vector.tensor_tensor(m4[:], qo, cos_i, OP.mult)
                        with nc.allow_low_precision(reason="roped q/k to fp8 for DR in_proj"):
                            nc.vector.tensor_tensor(dst[:, i, :], m1[:], m2[:], OP.subtract)
                            nc.vector.tensor_tensor(dst[:, 4 + i, :], m3[:], m4[:], OP.add)

            # ---- v' (fp8 DoubleRow): natural [t, e'] layout + ones col ----
            def vprime_chunk(ec, tog):
                pss = [ps_mm(), ps_mm(),
                       psum.tile([P, 512], F32, tag="sc", bufs=2, name=_nm("psv")),
                       psum.tile([P, 512], F32, tag="pv", bufs=2, name=_nm("psv"))]
                for vp in range(4):
                    wt2 = w2p.tile([P, 2, 512], F8, tag="w2", name=_nm("wt2"))
                    dma_w(wt2[:], wvT_d[vp][:, :, ec * 512:(ec + 1) * 512])
                    for tl in range(4):
                        tg = tog * 4 + tl
                        s2, ttt = divmod(tg, 4)
                        nc.tensor.matmul(pss[tl][:],
                                         lhsT=v8[s2][:, 2 * vp:2 * vp + 2, ttt * P:(ttt + 1) * P],
                                         rhs=wt2[:], start=(vp == 0), stop=False, perf_mode=DR)
                for tl in range(4):
                    nc.tensor.matmul(pss[tl][:], lhsT=onesr[:], rhs=bvrow[:, ec * 512:(ec + 1) * 512],
                                     start=False, stop=True)
                for tl in range(4):
                    kt = tog * 4 + tl
                    nc.vector.tensor_scalar(
                        out=vsb[ec][:, kt, :, 0:64],
                        in0=pss[tl][:].rearrange("p (h c) -> p h c", h=8),
                        scalar1=0.125, scalar2=None, op0=OP.mult)

            # ---- in_proj (fp8 DoubleRow), scores, exp, PV ----
            def in_proj_fo(fo):
                qf = qkp.tile([P, T], DTM, tag="qk", name=_nm("qf"))
                kf = qkp.tile([P, T], DTM, tag="qk", name=_nm("kf"))
                for jj, wd, s1, bb, dst in ((0, wq_d, 0.015625, bq, qf),
                                            (1, wk_d, 0.125, bk, kf)):
                    wt = w1p.tile([P, 8, P], F8, tag="w1", name=_nm("wt"))
                    dma_w(wt[:], wd[fo])
                    for qc in range(2):
                        ps = ps_mm()
                        for kp in range(4):
                            nc.tensor.matmul(ps[:], lhsT=wt[:, 2 * kp:2 * kp + 2, :],
                                             rhs=qk8[qc][jj][:, 2 * kp:2 * kp + 2, :],
                                             start=(kp == 0), stop=(kp == 3), perf_mode=DR)
                        nc.vector.tensor_scalar(out=dst[:, qc * S:(qc + 1) * S], in0=ps[:],
                                                scalar1=s1, scalar2=bb[:, fo:fo + 1],
                                                op0=OP.mult, op1=OP.add)
                return qf, kf

            def scores_fo(fo, qc, qf, kf):
                pTs2 = [ptsp.tile([P, 8, S], DTM, tag="pts", name=_nm("pts"))
                        for _ in range(2)]
                for kcp in range(4):
                    pp = [psum.tile([P, T], F32, tag="sc", bufs=2, name=_nm("pssc"))
                          for _ in range(2)]
                    for ki in range(2):
                        kc = 2 * kcp + ki
                        for hp in range(2):
                            poff = 64 * hp
                            nc.tensor.matmul(
                                pp[hp][:, ki * S:(ki + 1) * S],
                                lhsT=kf[poff:poff + 64, kc * P:(kc + 1) * P],
                                rhs=qf[poff:poff + 64, qc * S:(qc + 1) * S],
                                start=True, stop=True, tile_position=(poff, 0))
                    for hp in range(2):
                        nc.scalar.activation(pTs2[hp][:, 2 * kcp:2 * kcp + 2, :], pp[hp][:], AF.Exp)
                return pTs2

            def pv_fo(fo, qc, pTs2):
                """PV for both heads of fo, then the per-head normalize chain."""
                ops = []
                for hp in range(2):
                    h = 2 * fo + hp
                    vt = vsb[h // 8]
                    hh = h % 8
                    op = psum.tile([P, 512], F32, tag="pv", bufs=2, name=_nm("pspv"))
                    for kc in range(8):
                        nc.tensor.matmul(op[0:65, :], lhsT=vt[:, kc, hh, :],
                                         rhs=pTs2[hp][:, kc, :], start=(kc == 0), stop=(kc == 7))
                    ops.append(op)

                rcb = rbp.tile([65, 1024], DTM, tag="rcb", name=_nm("rcb"), bufs=2)
                for hp, op in enumerate(ops):
                    poff = 64 * hp
                    nc.vector.tensor_copy(oT[qc][poff:poff + 64, fo, :], op[0:64, :])
                    # custom-DVE recip reads garbage from PSUM on HW: stage
                    # the denominator row through SBUF first
                    dsb = rbp.tile([65, 512], F32, tag="dsb", name=_nm("dsb"), bufs=2)
                    nc.vector.tensor_copy(dsb[64:65, :], op[64:65, :])
                    rcf = rbp.tile([65, 512], F32, tag="rcf", name=_nm("rcf"), bufs=2)
                    nc.vector.reciprocal_approx_fast(out=rcf[64:65, :], in_=dsb[64:65, :])
                    with nc.allow_low_precision(reason="softmax 1/denom to bf16"):
                        nc.vector.tensor_copy(rcb[64:65, hp * 512:(hp + 1) * 512], rcf[64:65, :])
                if fo == 0 and qc == 0:
                    dbg_dump("dbg_rcb", rcb[64:65, :], [1, 1024], DTM)
                # rank-1 broadcast of both heads' 1/d into one psum bank
                bp = psum.tile([P, 512], F32, tag="pv", bufs=2, name=_nm("psbc"))
                for hp in range(2):
                    nc.tensor.matmul(bp[64 * hp:64 * hp + 64, :], lhsT=onesb[64:65, 0:64],
                                     rhs=rcb[64:65, hp * 512:(hp + 1) * 512],
                                     start=True, stop=True)
                with nc.allow_low_precision(reason="normalized o to fp8 for DR out_proj"):
                    nc.vector.tensor_tensor(oT8[qc][:, fo, :], oT[qc][:, fo, :], bp[:], OP.mult)

            # ================= emission =================
            qkv_block(0, range(16))
            rope_stream(0)
            load_xt(1)
            qkv_block(1, range(16))
            rope_stream(1)

            ipj = in_proj_fo(0)
            ipj2 = in_proj_fo(1)
            cur = [(qc, scores_fo(0, qc, *ipj)) for qc in range(2)]
            dbg_dump("dbg_qpre", qkv[0][0][:], [P, 8, S], DTM)
            dbg_dump("dbg_qrope", qk8[0][0][:], [P, 8, S], F8)
            dbg_dump("dbg_qf", ipj[0][:], [P, T], DTM)
            dbg_dump("dbg_kf", ipj[1][:], [P, T], DTM)
            dbg_dump("dbg_pts", cur[0][1][0][:], [P, 8, S], DTM)

            # v-path work: emitted here so its psum slot tenancy precedes the
            # attention loop's (slot rotation follows emission order); runs as
            # PE filler under the ACT exp stream
            qkv_block(0, range(16, 24))
            qkv_block(1, range(16, 24))
            for ec in range(2):
                for tog in range(2):
                    vprime_chunk(ec, tog)

            dbg_dump("dbg_v8", v8[0][:], [P, 8, S], F8)
            dbg_dump("dbg_vsb", vsb[0][:], [P, 8, 8, 65], DTM)

            for fo in range(8):
                ipj3 = in_proj_fo(fo + 2) if fo < 6 else None
                nxt = [(qc, scores_fo(fo + 1, qc, *ipj2)) for qc in range(2)] if fo < 7 else []
                for qc, pTs2 in cur:
                    pv_fo(fo, qc, pTs2)
                cur = nxt
                ipj, ipj2 = ipj2, ipj3

            dbg_dump("dbg_oT", oT[0][:], [P, 8, S], DTM)
            dbg_dump("dbg_oT8", oT8[0][:], [P, 8, S], F8)

            # ---- out_proj + per-stream MLP, dense PE (unchanged from v1) ----
            def out_proj(qc, yq):
                for eo in range(8):
                    wt = w1p.tile([P, 8, P], F8, tag="w1", name=_nm("wt"))
                    dma_w(wt[:], wo_d[eo])
                    ps = ps_mm()
                    for fp in range(4):
                        nc.tensor.matmul(ps[:], lhsT=wt[:, 2 * fp:2 * fp + 2, :],
                                         rhs=oT8[qc][:, 2 * fp:2 * fp + 2, :],
                                         start=(fp == 0), stop=(fp == 3), perf_mode=DR)
                    with nc.allow_low_precision(reason="y to fp8 for DR up"):
                        nc.vector.tensor_scalar(out=yq[:, eo, :], in0=ps[:], scalar1=0.125,
                                                scalar2=bo[:, eo:eo + 1], op0=OP.mult, op1=OP.add)

            def mlp_up_group(s, fo, yq, hts):
                wt = w1p.tile([P, 8, P], F8, tag="w1", name=_nm("wt"))
                dma_w(wt[:], w1_d[s][fo])
                ps = ps_mm()
                for kp in range(4):
                    nc.tensor.matmul(ps[:], lhsT=wt[:, 2 * kp:2 * kp + 2, :],
                                     rhs=yq[:, 2 * kp:2 * kp + 2, :],
                                     start=(kp == 0), stop=(kp == 3), perf_mode=DR)
                with nc.allow_low_precision(reason="mlp hidden to fp8 for DR down"):
                    nc.scalar.activation(hts[fo // 8][:, fo % 8, :], ps[:], AF.Gelu_apprx_tanh,
                                         bias=b1[s][:, fo:fo + 1], scale=0.125)

            def mlp_down_ec(s, hts, ec):
                zn2 = big.tile([P, 4, 512], F32, tag="qkvqk", bufs=4, name=_nm("zn2"))
                nc.sync.dma_start(
                    out=zn2[:],
                    in_=z_src[s][:, ec * 512:(ec + 1) * 512].rearrange("(to p) d -> p to d", p=P))
                pss = [ps_mm(), ps_mm(),
                       psum.tile([P, 512], F32, tag="sc", bufs=2, name=_nm("psg")),
                       psum.tile([P, 512], F32, tag="pv", bufs=2, name=_nm("psg"))]
                for cp in range(16):
                    wt2 = w2p.tile([P, 2, 512], F8, tag="w2", name=_nm("wt2"))
                    dma_w(wt2[:], w2T_d[s][cp][:, :, ec * 512:(ec + 1) * 512])
                    fo = 2 * cp
                    for tl in range(4):
                        nc.tensor.matmul(pss[tl][:],
                                         lhsT=hts[fo // 8][:, fo % 8:fo % 8 + 2, tl * P:(tl + 1) * P],
                                         rhs=wt2[:], start=(cp == 0), stop=False, perf_mode=DR)
                for tl in range(4):
                    nc.tensor.matmul(pss[tl][:], lhsT=onesr[:], rhs=b2row[s][:, ec * 512:(ec + 1) * 512],
                                     start=False, stop=True)
                for tl in range(4):
                    t2 = rs.tile([P, 512], F32, tag="rs", name=_nm("ost"))
                    nc.vector.scalar_tensor_tensor(t2[:], pss[tl][:], 0.125, zn2[:, tl, :],
                                                   OP.mult, OP.add)
                    nc.sync.dma_start(out=out_d[s][tl * P:(tl + 1) * P, ec * 512:(ec + 1) * 512], in_=t2[:])

            # y reuses the v8 ring (dead after v'); hts the qk8 ring (dead
            # after the last in_proj)
            yT = {}
            for qc in (0, 1):
                y = big.tile([P, 8, S], F8, tag="v8", bufs=2, name=_nm("yf"))
                out_proj(qc, y)
                yT[qc] = y
            dbg_dump("dbg_y", yT[0][:], [P, 8, S], F8)
            for s in (0, 1):
                hts = [big.tile([P, 8, S], F8, tag="qk8", bufs=4, name=_nm("htf")) for _ in range(4)]
                for fo in range(32):
                    mlp_up_group(s, fo, yT[s], hts)
                mlp_down_ec(s, hts, 0)
                mlp_down_ec(s, hts, 1)

    nc.finalize()
    return nc


def _to4(WT, dt):
    """WT [Din, Eout] -> [Eout/128, 128p, Din/128, 128e] tiles for lhsT DMA."""
    din, eout = WT.shape
    a = WT.reshape(din // P, P, eout // P, P)       # [ko, p, eo, e]
    return np.ascontiguousarray(a.transpose(2, 1, 0, 3).astype(dt))


def _bias_part(b, n_tiles):
    return np.ascontiguousarray(b.reshape(n_tiles, P).T)


def _prep_shared(inputs):
    f32 = lambda x: np.ascontiguousarray(np.asarray(x, dtype=np.float32))
    np8 = mybir.dt.np(F8)
    perm = np.concatenate([np.arange(0, D, 2), np.arange(1, D, 2)])

    shared = {}
    for s, (wn, bn) in enumerate((("qkv_state_w", "qkv_state_b"), ("qkv_action_w", "qkv_action_b"))):
        w = f32(inputs[wn])
        b = f32(inputs[bn])
        wp = np.concatenate([w[0:D][perm], w[D:2 * D][perm], w[2 * D:3 * D]], axis=0)
        bp = np.concatenate([b[0:D][perm], b[D:2 * D][perm], b[2 * D:3 * D]])
        shared[f"wqkv{s}"] = _to4(wp.T * 8.0, np8)
        shared[f"bqkv{s}"] = _bias_part(bp, 24)

    in_w = f32(inputs["attn_in_w"])
    in_b = f32(inputs["attn_in_b"])
    wq, wk, wv = in_w[0:D], in_w[D:2 * D], in_w[2 * D:3 * D]
    bq_, bk_, bv_ = in_b[0:D], in_b[D:2 * D], in_b[2 * D:3 * D]
    scale = 1.0 / math.sqrt(HD)
    shared["wq"] = _to4(wq[:, perm].T * 8.0, np8)        # raw x8; 1/8 attn scale in epilogue
    shared["bq"] = _bias_part(bq_ * scale, 8)
    shared["wk"] = _to4(wk[:, perm].T * 8.0, np8)
    shared["bk"] = _bias_part(bk_, 8)
    wvT = wv.T * 8.0                                      # [Din, D]
    shared["wvT"] = np.ascontiguousarray(
        wvT.reshape(4, 2, P, D).transpose(0, 2, 1, 3).astype(np8))
    shared["bvrow"] = np.ascontiguousarray((bv_ * 8.0)[None, :].astype(NPM))
    shared["wo"] = np.ascontiguousarray(
        (_to4(f32(inputs["attn_out_w"]).T, np.float32) * 8.0).astype(np8))
    shared["bo"] = _bias_part(f32(inputs["attn_out_b"]), 8)
    for s, pre in enumerate(("mlp_state", "mlp_action")):
        w1a = _to4(f32(inputs[f"{pre}_w1"]).T, np.float32) * 8.0
        shared[f"w1{s}"] = np.ascontiguousarray(w1a.astype(np8))
        shared[f"b1{s}"] = _bias_part(f32(inputs[f"{pre}_b1"]), 32)

    inv = np.exp(-math.log(MAX_LEN) * np.arange(0, D, 2, dtype=np.float64) / D)
    theta = inv[:, None] * np.arange(S, dtype=np.float64)[None, :]   # [i, t]
    cosT = np.cos(theta).astype(np.float32)
    sinT = np.sin(theta).astype(np.float32)
    shared["cosT"] = np.ascontiguousarray(cosT.reshape(4, P, S).transpose(1, 0, 2).reshape(P, 2048).astype(NPM))
    shared["sinT"] = np.ascontiguousarray(sinT.reshape(4, P, S).transpose(1, 0, 2).reshape(P, 2048).astype(NPM))
    shared["onesr"] = np.ones((1, P), NPM)
    shared["onesb"] = np.ones((P, 512), NPM)
    return shared


def _prep_in_maps(inputs):
    f32 = lambda x: np.ascontiguousarray(np.asarray(x, dtype=np.float32))
    np8 = mybir.dt.np(F8)
    shared = _prep_shared(inputs)
    state_z = f32(inputs["state_z"])
    action_z = f32(inputs["action_z"])
    e = f32(inputs["e"])
    w2T = [f32(inputs["mlp_state_w2"]).T, f32(inputs["mlp_action_w2"]).T]   # [FF, D]
    b2 = [f32(inputs["mlp_state_b2"]), f32(inputs["mlp_action_b2"])]
    in_maps = []
    for b in range(B):
        shift = e[b, 0, 0:D]
        scl1p = 1.0 + e[b, 0, D:2 * D]
        res = e[b, 0, 2 * D:3 * D]
        m = dict(shared)
        m["sz"] = state_z[b]
        m["az"] = action_z[b]
        for s, z in ((0, state_z[b]), (1, action_z[b])):
            xt = (scl1p[None, :] * z + shift[None, :]).T      # [D, S]
            m[f"xt{s}"] = np.ascontiguousarray(xt.reshape(8, P, S).astype(np8))
            w2r = (w2T[s] * res[None, :] * 8.0).reshape(16, 2, P, D).transpose(0, 2, 1, 3)
            m[f"w2T{s}"] = np.ascontiguousarray(w2r.astype(np8))
            m[f"b2row{s}"] = np.ascontiguousarray((b2[s] * res * 8.0)[None, :].astype(NPM))
        in_maps.append(m)
    return in_maps


def _run(inputs, trace=False, trace_kwargs=None, tmpdir=None, debug=False):
    key = f"nc{debug}"
    if key not in _BUILD_CACHE:
        _BUILD_CACHE[key] = _build_nc(debug=debug)
    nc = _BUILD_CACHE[key]
    in_maps = _prep_in_maps(inputs)
    kw = {}
    if trace:
        kw = dict(trace=True, trace_kwargs=trace_kwargs or {})
    if tmpdir is not None:
        kw["tmpdir"] = tmpdir
    return run_bass_kernel_spmd(nc, in_maps, list(range(N_CORES)), **kw)


def kernel(**inputs):
    res = _run(inputs)
    state = np.stack([res.results[b]["state_out"] for b in range(B)])
    action = np.stack([res.results[b]["action_out"] for b in range(B)])
    return (state, action)


def kernel_timed(tmpdir=None, **inputs):
    """Returns ((state, action), exec_time_ns) using the NTFF profile path."""
    res = _run(inputs, trace=True, tmpdir=tmpdir)
    state = np.stack([res.results[b]["state_out"] for b in range(B)])
    action = np.stack([res.results[b]["action_out"] for b in range(B)])
    return (state, action), res.exec_time_ns
